# revision 1
# baseline (speedup 1.0000x reference)
"""CrossAttentionFusion kernel for Trainium2 (8 NeuronCores, Bass/Tile).

Computation (matches the reference nn.Module):
  image_proj = relu(BN(1x1conv(image_features, image_w)))   # (B,128,H,W)
  lidar_proj = relu(BN(1x1conv(lidar_features, lidar_w)))   # (B,128,H,W)
  per (batch, 2048-pixel chunk): q = image_proj, k = v = lidar_proj
  attn_out = softmax(q k^T / sqrt(128)) @ k
  out = w0 * image_proj + w1 * attn_out,  w = softmax(modality_weights)

Sharding: the 16 independent (batch, chunk) attention problems are
distributed 2-per-core across 8 cores; each core also computes the
projections for its own pixels.  Host gathers the 8 outputs.

Per-core layout / engine plan:
  - Projections and scores run in f32r (1 cycle/row on the PE for 512-wide
    moving dim, near-f32 precision -- the peaked softmax needs accurate
    scores).  BN is folded into an affine on the host; relu on ACT writes
    f32r; w0 is folded into the image BN affine.
  - Scores are k-major: sT[kpix, q] so AV needs no transposed exp.
  - AV runs in bf16: exp writes bf16 tiles, K pixel-major bf16 tiles (AV
    lhsT) come from a GPSIMD bf16 downcast of kT + one SBUF->SBUF DMA XBAR
    transpose per unit, keeping the PE free of transposes.
  - exp is split across three engines: ACT (true exp -> bf16), DVE and
    GPSIMD (Schraudolph bit-trick: y=int16(s*a+b) reinterpreted as bf16,
    ~3% weight error that cancels through the shared softmax denominator).
  - softmax denominator: S = sum_i ET_i accumulated on DVE in bf16
    (2-byte fast mode), partition-reduced+broadcast by one GPSIMD
    partition_all_reduce, reciprocal on DVE.
  - res = (po * w1) * linv + qT via one DVE scalar_tensor_tensor + one add.
"""

import math
import os
import sys
from contextlib import ExitStack

import numpy as np

sys.path.insert(0, "/opt/trn_rl_repo")

import concourse.bass as bass  # noqa: E402
import concourse.tile as tile  # noqa: E402
from concourse import bacc, bass_isa, mybir  # noqa: E402
from concourse.bass import ds, ts  # noqa: E402
from concourse.bass_utils import run_bass_kernel_spmd  # noqa: E402

F32 = mybir.dt.float32
F32R = mybir.dt.float32r
BF16 = mybir.dt.bfloat16
I16 = mybir.dt.int16

B, CL, CI, CO = 2, 256, 512, 128
H = W = 128
P = H * W                    # 16384 pixels per batch
CHUNK = 2048                 # attention chunk (pixels)
NCH = P // CHUNK             # 8 chunks per batch
NCORES = 8
UPC = (B * NCH) // NCORES    # units (b,chunk) per core = 2
EPS = 1e-5
QB = 1024                    # q-block width (2 matmul halves of 512)
NQB = CHUNK // QB            # 2
KSL = CHUNK // 128           # 16 k-pixel slices per chunk
NCI_IMG = CI // 128          # 4 contraction slices for image proj
NCI_LID = CL // 128          # 2 for lidar proj

# exp engine per k-slice index: A=ACT true exp, D=DVE Schraudolph.
# (GPSIMD cannot read PSUM, so it gets only SBUF-side work.)
if os.environ.get("K_EXP_MODE", "mixed") == "act":
    EXP_ENG = ["A"] * 16
else:
    EXP_ENG = ["A", "A", "D", "A", "A", "A", "D", "A",
               "A", "A", "D", "A", "A", "A", "A", "A"]
ADD_LAG = int(os.environ.get("K_ADD_LAG", "3"))   # S-add lags exp by this many
K_RESADD = os.environ.get("K_RESADD", "gp")       # gp | dve
K_KTB = os.environ.get("K_KTB", "gp")             # gp | dve
K_KP = os.environ.get("K_KP", "pe")               # pe | dma
K_ET_BUFS = int(os.environ.get("K_ET_BUFS", str(KSL + 2)))
K_XI_BUFS = int(os.environ.get("K_XI_BUFS", "6"))
LOOKAHEAD = 2                # AV matmuls lag scores by this many slices

# Schraudolph constants for bf16-bits-in-int16 exp approximation
# (DVE/GPSIMD float->int conversion truncates; c tuned for that):
#   exp(x) ~= bitcast_bf16(int16(x * 128/ln2 + (127*128 - C)))
SCH_A = 128.0 / math.log(2.0)
SCH_C = 5.0

_PROGRAM = None              # compiled Bass program, built once per process
LAST_RESULTS = None          # BassKernelResults of the last kernel() call


def _build_program():
    nc = bacc.Bacc("TRN2", target_bir_lowering=False, debug=False,
                   num_devices=NCORES)

    # Per-core DRAM inputs (pre-sharded on host).
    ximg = nc.dram_tensor("ximg", [UPC, NCI_IMG, 128, CHUNK], F32R,
                          kind="ExternalInput").ap()
    xlid = nc.dram_tensor("xlid", [UPC, NCI_LID, 128, CHUNK], F32R,
                          kind="ExternalInput").ap()
    wimg = nc.dram_tensor("wimg", [NCI_IMG, 128, CO], F32R,
                          kind="ExternalInput").ap()
    wlid = nc.dram_tensor("wlid", [NCI_LID, 128, CO], F32R,
                          kind="ExternalInput").ap()
    img_scale = nc.dram_tensor("img_scale", [CO, 1], F32, kind="ExternalInput").ap()
    img_bias = nc.dram_tensor("img_bias", [CO, 1], F32, kind="ExternalInput").ap()
    lid_scale = nc.dram_tensor("lid_scale", [CO, 1], F32, kind="ExternalInput").ap()
    lid_bias = nc.dram_tensor("lid_bias", [CO, 1], F32, kind="ExternalInput").ap()
    escale = nc.dram_tensor("escale", [128, 1], F32, kind="ExternalInput").ap()
    w1t = nc.dram_tensor("w1t", [128, 1], F32, kind="ExternalInput").ap()
    sch_a = nc.dram_tensor("sch_a", [128, 1], F32, kind="ExternalInput").ap()
    sch_b = nc.dram_tensor("sch_b", [128, 1], F32, kind="ExternalInput").ap()
    identb = nc.dram_tensor("identb", [128, 128], BF16, kind="ExternalInput").ap()
    onesb = nc.dram_tensor("onesb", [128, 128], BF16, kind="ExternalInput").ap()
    y = nc.dram_tensor("y", [UPC, CO, CHUNK], F32, kind="ExternalOutput").ap()

    with tile.TileContext(nc) as tc, ExitStack() as ctx:
        const = ctx.enter_context(tc.tile_pool(name="const", bufs=1))
        xi_pool = ctx.enter_context(tc.tile_pool(name="xi", bufs=K_XI_BUFS))
        xl_pool = ctx.enter_context(tc.tile_pool(name="xl", bufs=2))
        proj_pool = ctx.enter_context(tc.tile_pool(name="proj", bufs=2))
        ktb_pool = ctx.enter_context(tc.tile_pool(name="ktb", bufs=2))
        kp_pool = ctx.enter_context(tc.tile_pool(name="kp", bufs=2))
        et_pool = ctx.enter_context(tc.tile_pool(name="et", bufs=K_ET_BUFS))
        s_pool = ctx.enter_context(tc.tile_pool(name="s", bufs=4))
        misc_pool = ctx.enter_context(tc.tile_pool(name="misc", bufs=2))
        res_pool = ctx.enter_context(tc.tile_pool(name="res", bufs=2))
        # PSUM: scores/proj ring 2x[128,1024] (4 banks) + AV po 2x (4 banks)
        mm_psum = ctx.enter_context(tc.tile_pool(name="mmps", bufs=2, space="PSUM"))
        av_psum = ctx.enter_context(tc.tile_pool(name="avps", bufs=2, space="PSUM"))

        # constants
        wimg_t = const.tile([128, NCI_IMG * CO], F32R)
        for ci in range(NCI_IMG):
            nc.sync.dma_start(wimg_t[:, ts(ci, CO)], wimg[ci])
        wlid_t = const.tile([128, NCI_LID * CO], F32R)
        for ci in range(NCI_LID):
            nc.sync.dma_start(wlid_t[:, ts(ci, CO)], wlid[ci])
        img_s = const.tile([128, 1], F32)
        nc.sync.dma_start(img_s[:], img_scale)
        img_b = const.tile([128, 1], F32)
        nc.sync.dma_start(img_b[:], img_bias)
        lid_s = const.tile([128, 1], F32)
        nc.sync.dma_start(lid_s[:], lid_scale)
        lid_b = const.tile([128, 1], F32)
        nc.sync.dma_start(lid_b[:], lid_bias)
        esc = const.tile([128, 1], F32)
        nc.sync.dma_start(esc[:], escale)
        w1s = const.tile([128, 1], F32)
        nc.sync.dma_start(w1s[:], w1t)
        sch_as = const.tile([128, 1], F32)
        nc.sync.dma_start(sch_as[:], sch_a)
        sch_bs = const.tile([128, 1], F32)
        nc.sync.dma_start(sch_bs[:], sch_b)
        ident_t = const.tile([128, 128], BF16)
        nc.sync.dma_start(ident_t[:], identb)
        ones_t = const.tile([128, 128], BF16)
        nc.sync.dma_start(ones_t[:], onesb)

        for u in range(UPC):
            # ---- load unit inputs ----
            xl = []
            for ci in range(NCI_LID):
                t = xl_pool.tile([128, CHUNK], F32R, name=f"xl_{u}_{ci}", tag="xl")
                for hh in range(2):
                    nc.sync.dma_start(t[:, ts(hh, QB)], xlid[u, ci, :, ts(hh, QB)])
                xl.append(t)
            xi = []
            for ci in range(NCI_IMG):
                t = xi_pool.tile([128, CHUNK], F32R, name=f"xi_{u}_{ci}", tag="xi")
                for hh in range(2):
                    nc.sync.dma_start(t[:, ts(hh, QB)], ximg[u, ci, :, ts(hh, QB)])
                xi.append(t)

            # ---- projections (channel-major), QB-wide PSUM, halves of 512 ----
            qT = proj_pool.tile([128, CHUNK], F32R, name=f"qT_{u}", tag="qT")
            kT = proj_pool.tile([128, CHUNK], F32R, name=f"kT_{u}", tag="kT")
            for qb in range(NQB):
                ps2 = mm_psum.tile([128, QB], F32, name=f"psl_{u}_{qb}", tag="mm")
                for h in range(QB // 512):
                    for ci in range(NCI_LID):
                        nc.tensor.matmul(ps2[:, ts(h, 512)], wlid_t[:, ts(ci, CO)],
                                         xl[ci][:, ds(qb * QB + h * 512, 512)],
                                         start=(ci == 0), stop=(ci == NCI_LID - 1))
                nc.scalar.activation(kT[:, ts(qb, QB)], ps2[:],
                                     mybir.ActivationFunctionType.Relu,
                                     bias=lid_b[:], scale=lid_s[:])

            # ---- K pixel-major bf16: GPSIMD downcast + PE transpose ----
            kTb = ktb_pool.tile([128, CHUNK], BF16, name=f"kTb_{u}", tag="ktb")
            keng = nc.gpsimd if K_KTB == "gp" else nc.vector
            for qb in range(NQB):
                keng.tensor_copy(kTb[:, ts(qb, QB)], kT[:, ts(qb, QB)])
            kp = kp_pool.tile([128, KSL * 128], BF16, name=f"kp_{u}", tag="kp")
            for g in range(KSL // 8):
                pt = mm_psum.tile([128, 8 * 128], BF16,
                                  name=f"pt_{u}_{g}", tag="mm")
                for k in range(8):
                    nc.tensor.transpose(pt[:, ts(k, 128)],
                                        kTb[:, ds(g * 8 * 128 + k * 128, 128)],
                                        ident_t[:])
                nc.vector.tensor_copy(kp[:, ts(g, 8 * 128)], pt[:])

            for qb in range(NQB):
                ps = mm_psum.tile([128, QB], F32, name=f"psi_{u}_{qb}", tag="mm")
                for h in range(QB // 512):
                    for ci in range(NCI_IMG):
                        nc.tensor.matmul(ps[:, ts(h, 512)], wimg_t[:, ts(ci, CO)],
                                         xi[ci][:, ds(qb * QB + h * 512, 512)],
                                         start=(ci == 0), stop=(ci == NCI_IMG - 1))
                nc.scalar.activation(qT[:, ts(qb, QB)], ps[:],
                                     mybir.ActivationFunctionType.Relu,
                                     bias=img_b[:], scale=img_s[:])

            # ---- attention, one q-block at a time ----
            res_u = res_pool.tile([128, CHUNK], F32, name=f"res_{u}", tag="res")
            for qb in range(NQB):
                po = av_psum.tile([128, QB], F32, name=f"po_{u}_{qb}", tag="av")
                # ping-pong S so DVE adds never read+write the same tile
                Ss = [s_pool.tile([128, QB], BF16, name=f"S{t}_{u}_{qb}", tag="S")
                      for t in range(2)]
                ets = [None] * KSL

                def s_add(i):
                    # S_{i} = S_{i-1} + et_i  (i >= 1); S_1 = et_0 + et_1
                    src0 = ets[0][:] if i == 1 else Ss[i % 2][:]
                    nc.vector.tensor_add(Ss[(i + 1) % 2][:], src0, ets[i][:])

                for i in range(KSL + max(LOOKAHEAD, ADD_LAG + 1)):
                    if i < KSL:
                        ps = mm_psum.tile([128, QB], F32,
                                          name=f"pss_{u}_{qb}_{i}", tag="mm")
                        for h in range(QB // 512):
                            nc.tensor.matmul(ps[:, ts(h, 512)], kT[:, ts(i, 128)],
                                             qT[:, ds(qb * QB + h * 512, 512)],
                                             start=True, stop=True)
                        et = et_pool.tile([128, QB], BF16,
                                          name=f"et_{u}_{qb}_{i}", tag="et")
                        eng = EXP_ENG[i]
                        if eng == "A":
                            nc.scalar.activation(et[:], ps[:],
                                                 mybir.ActivationFunctionType.Exp,
                                                 scale=esc[:])
                        else:
                            nc.vector.tensor_scalar(et[:].bitcast(I16), ps[:],
                                                    sch_as[:], sch_bs[:],
                                                    op0=mybir.AluOpType.mult,
                                                    op1=mybir.AluOpType.add)
                        ets[i] = et
                    a = i - ADD_LAG
                    if 1 <= a < KSL:
                        s_add(a)
                    j = i - LOOKAHEAD
                    if 0 <= j < KSL:
                        for h in range(QB // 512):
                            nc.tensor.matmul(po[:, ts(h, 512)], kp[:, ds(j * 128, 128)],
                                             ets[j][:, ts(h, 512)],
                                             start=(j == 0), stop=(j == KSL - 1))
                S = Ss[KSL % 2]
                # denominator: PE broadcast-sum ones^T @ S, then 1/x
                pl = mm_psum.tile([128, QB], F32, name=f"pl_{u}_{qb}", tag="mm")
                for h in range(QB // 512):
                    nc.tensor.matmul(pl[:, ts(h, 512)], ones_t[:],
                                     S[:, ts(h, 512)], start=True, stop=True)
                linv = misc_pool.tile([128, QB], F32, name=f"linv_{u}_{qb}",
                                      tag="linv")
                nc.vector.reciprocal_approx_fast(linv[:], pl[:])
                # res = (po * w1) * linv + qT   (qT already carries w0)
                tmp = misc_pool.tile([128, QB], F32, name=f"tmp_{u}_{qb}",
                                     tag="tmp")
                nc.vector.scalar_tensor_tensor(tmp[:], po[:], w1s[:], linv[:],
                                               op0=mybir.AluOpType.mult,
                                               op1=mybir.AluOpType.mult)
                nc.vector.tensor_add(res_u[:, ts(qb, QB)], tmp[:],
                                     qT[:, ts(qb, QB)])
            nc.sync.dma_start(y[u], res_u[:])

    nc.compile()
    return nc


def _shard_inputs(inputs):
    """Build the 8 per-core input maps from the full input dict."""
    mw = np.asarray(inputs["modality_weights"], np.float64)
    e = np.exp(mw - mw.max())
    w = (e / e.sum()).astype(np.float64)
    w0, w1 = float(w[0]), float(w[1])

    def bn_fold(gamma, beta, mean, var, mul):
        g = np.asarray(gamma, np.float64)
        b = np.asarray(beta, np.float64)
        m = np.asarray(mean, np.float64)
        v = np.asarray(var, np.float64)
        scale = g / np.sqrt(v + EPS) * mul
        bias = (b - m * g / np.sqrt(v + EPS)) * mul
        return (scale.astype(np.float32).reshape(CO, 1),
                bias.astype(np.float32).reshape(CO, 1))

    i_s, i_b = bn_fold(inputs["image_gamma"], inputs["image_beta"],
                       inputs["image_mean"], inputs["image_var"], w0)
    l_s, l_b = bn_fold(inputs["lidar_gamma"], inputs["lidar_beta"],
                       inputs["lidar_mean"], inputs["lidar_var"], 1.0)

    # weight slices, pre-transposed for lhsT ([cin_slice, cout])
    wi = np.ascontiguousarray(
        np.asarray(inputs["image_w"], np.float32).T.reshape(NCI_IMG, 128, CO))
    wl = np.ascontiguousarray(
        np.asarray(inputs["lidar_w"], np.float32).T.reshape(NCI_LID, 128, CO))

    escv = 1.0 / (w0 * math.sqrt(CO))
    esc = np.full((128, 1), escv, np.float32)
    w1v = np.full((128, 1), w1, np.float32)
    sch_av = np.full((128, 1), escv * SCH_A, np.float32)
    sch_bv = np.full((128, 1), 127.0 * 128.0 - SCH_C, np.float32)
    identb = np.eye(128, dtype=mybir.dt.np(BF16))
    onesb = np.ones((128, 128), dtype=mybir.dt.np(BF16))

    # full features reshaped to (B, C, nchunks, 2048)
    img = np.asarray(inputs["image_features"], np.float32).reshape(B, CI, NCH, CHUNK)
    lid = np.asarray(inputs["lidar_features"], np.float32).reshape(B, CL, NCH, CHUNK)

    in_maps = []
    for core in range(NCORES):
        ximg = np.empty((UPC, NCI_IMG, 128, CHUNK), np.float32)
        xlid = np.empty((UPC, NCI_LID, 128, CHUNK), np.float32)
        for ul in range(UPC):
            un = core * UPC + ul
            b, c = un // NCH, un % NCH
            ximg[ul] = img[b, :, c, :].reshape(NCI_IMG, 128, CHUNK)
            xlid[ul] = lid[b, :, c, :].reshape(NCI_LID, 128, CHUNK)
        in_maps.append({
            "ximg": ximg, "xlid": xlid, "wimg": wi, "wlid": wl,
            "img_scale": i_s, "img_bias": i_b,
            "lid_scale": l_s, "lid_bias": l_b,
            "escale": esc, "w1t": w1v, "sch_a": sch_av, "sch_b": sch_bv,
            "identb": identb, "onesb": onesb,
        })
    return in_maps


def kernel(**inputs) -> np.ndarray:
    global _PROGRAM, LAST_RESULTS
    if _PROGRAM is None:
        _PROGRAM = _build_program()
    nc = _PROGRAM

    in_maps = _shard_inputs(inputs)
    trace = os.environ.get("BASS_KERNEL_TRACE", "0") == "1"
    tmpdir = os.environ.get("BASS_KERNEL_TRACE_DIR") or None
    if tmpdir:
        os.makedirs(tmpdir, exist_ok=True)
    results = run_bass_kernel_spmd(nc, in_maps, core_ids=list(range(NCORES)),
                                   trace=trace, tmpdir=tmpdir)
    LAST_RESULTS = results

    out = np.empty((B, CO, H, W), np.float32)
    outv = out.reshape(B, CO, NCH, CHUNK)
    for core in range(NCORES):
        yc = results.results[core]["y"]
        for ul in range(UPC):
            un = core * UPC + ul
            b, c = un // NCH, un % NCH
            outv[b, :, c, :] = yc[ul]
    return out


if __name__ == "__main__":
    rng = np.random.default_rng(0)
    inputs = {
        "lidar_features": rng.standard_normal((B, CL, H, W), np.float32),
        "image_features": rng.standard_normal((B, CI, H, W), np.float32),
        "lidar_w": rng.standard_normal((CO, CL), np.float32) * np.sqrt(2.0 / CO),
        "lidar_gamma": np.ones(CO, np.float32),
        "lidar_beta": np.zeros(CO, np.float32),
        "lidar_mean": rng.standard_normal(CO).astype(np.float32) * 0.1,
        "lidar_var": rng.uniform(0.5, 1.5, CO).astype(np.float32),
        "image_w": rng.standard_normal((CO, CI), np.float32) * np.sqrt(2.0 / CO),
        "image_gamma": np.ones(CO, np.float32),
        "image_beta": np.zeros(CO, np.float32),
        "image_mean": rng.standard_normal(CO).astype(np.float32) * 0.1,
        "image_var": rng.uniform(0.5, 1.5, CO).astype(np.float32),
        "modality_weights": np.ones(2, np.float32),
    }
    out = kernel(**inputs)
    print("kernel out:", out.shape, out.dtype, float(np.abs(out).mean()))



# revision 2
# speedup vs baseline: 1.3001x; 1.3001x over previous
"""CrossAttentionFusion kernel for Trainium2 (8 NeuronCores, Bass/Tile).

Computation (matches the reference nn.Module):
  image_proj = relu(BN(1x1conv(image_features, image_w)))   # (B,128,H,W)
  lidar_proj = relu(BN(1x1conv(lidar_features, lidar_w)))   # (B,128,H,W)
  per (batch, 2048-pixel chunk): q = image_proj, k = v = lidar_proj
  attn_out = softmax(q k^T / sqrt(128)) @ k
  out = w0 * image_proj + w1 * attn_out,  w = softmax(modality_weights)

Sharding: the 16 independent (batch, chunk) attention problems are
distributed 2-per-core across 8 cores; each core computes the projections
for its own pixels.  Host gathers the 8 outputs.

v2 design (all-bf16 data path):
  - Inputs and conv weights are cast to bf16 on the host (BN scale folded
    into the weights, w0 folded into the image weights/bias); f32 PSUM
    accumulation keeps the projections accurate; ACT applies bias+relu and
    writes qTb/kTb directly as bf16 (no separate downcast pass).
  - Scores: sT[k,q] = kTb_slice.T @ qTb (bf16 in, f32 psum), exp split
    between ACT (true exp -> bf16) and DVE (Schraudolph int16 bit-trick).
  - Softmax denominator: S = sum_i et_i on DVE (bf16), partition-reduced
    and broadcast by one (1/w1)-matrix matmul, reciprocal on DVE gives
    linv = w1/L, so the output is just po*linv + qTb (2 DVE ops).
  - AV: kp (pixel-major bf16 via PE transposes) @ et, f32 psum accum.
  - DMAs: 2 packed const DMAs + 2 per (unit, modality) issued up front in
    consumption order; dummy warm-up matmuls keep the PE HAM warm through
    the DMA head; per-q-block output DMAs shrink the tail.
"""

import math
import os
import sys
from contextlib import ExitStack

import numpy as np

sys.path.insert(0, "/opt/trn_rl_repo")

import concourse.bass as bass  # noqa: E402
import concourse.tile as tile  # noqa: E402
from concourse import bacc, mybir  # noqa: E402
from concourse.bass import ds, ts  # noqa: E402
from concourse.bass_utils import run_bass_kernel_spmd  # noqa: E402

F32 = mybir.dt.float32
BF16 = mybir.dt.bfloat16
I16 = mybir.dt.int16

B, CL, CI, CO = 2, 256, 512, 128
H = W = 128
P = H * W                    # 16384 pixels per batch
CHUNK = 2048                 # attention chunk (pixels)
NCH = P // CHUNK             # 8 chunks per batch
NCORES = 8
UPC = (B * NCH) // NCORES    # units (b,chunk) per core = 2
EPS = 1e-5
QB = 1024                    # q-block width (2 matmul halves of 512)
NQB = CHUNK // QB            # 2
KSL = CHUNK // 128           # 16 k-pixel slices per chunk
NCI_I = CI // 128            # 4 contraction slices for image proj
NCI_L = CL // 128            # 2 for lidar proj

# exp engine per k-slice index: A=ACT true exp, D=DVE Schraudolph.
EXP_ENG = os.environ.get("K_EXP_ENG", "AADAAAAAADAAAAAA")
# S-add engine per add index 1..15: v=DVE, g=GPSIMD
ADD_ENG = os.environ.get("K_ADD_ENG", "vvvvvvvvvvvvvvvv")
ADD_LAG = int(os.environ.get("K_ADD_LAG", "3"))
LOOKAHEAD = int(os.environ.get("K_LOOKAHEAD", "2"))
K_WARM = int(os.environ.get("K_WARM", "8"))
# proj relu engine, 8 chars: u0[kh0,kh1,qh0,qh1], u1[...]  A=ACT, D=DVE
RELU_ENG = os.environ.get("K_RELU_ENG", "AAAAAAAA")
ET_BUFS = int(os.environ.get("K_ET_BUFS", "18"))

# Schraudolph constants for bf16-bits-in-int16 exp approximation
# (DVE float->int conversion truncates; C tuned for that):
#   exp(x) ~= bitcast_bf16(int16(x * 128/ln2 + (127*128 - C)))
SCH_A = 128.0 / math.log(2.0)
SCH_C = 5.0

# cf (f32 const) column indices
CF_IMG_B, CF_LID_B, CF_ESC, CF_SCHA, CF_SCHB = 0, 1, 2, 3, 4
# cb (bf16 const) column offsets
CB_WIMG, CB_WLID, CB_IDENT, CB_INVW1 = 0, 512, 768, 896

_PROGRAM = None              # compiled Bass program, built once per process
LAST_RESULTS = None          # BassKernelResults of the last kernel() call


def _build_program():
    nc = bacc.Bacc("TRN2", target_bir_lowering=False, debug=False,
                   num_devices=NCORES)

    cb = nc.dram_tensor("cb", [128, 1024], BF16, kind="ExternalInput").ap()
    cf = nc.dram_tensor("cf", [128, 8], F32, kind="ExternalInput").ap()
    # per-(unit,pixel-half) inputs, ci-major within the SBUF row
    xl = nc.dram_tensor("xl", [UPC, 2, 128, NCI_L, 1024], BF16,
                        kind="ExternalInput").ap()
    xi = nc.dram_tensor("xi", [UPC, 2, 128, NCI_I, 1024], BF16,
                        kind="ExternalInput").ap()
    y = nc.dram_tensor("y", [UPC, CO, CHUNK], F32, kind="ExternalOutput").ap()

    with tile.TileContext(nc) as tc, ExitStack() as ctx:
        const = ctx.enter_context(tc.tile_pool(name="const", bufs=1))
        xl_pool = ctx.enter_context(tc.tile_pool(name="xl", bufs=2))
        xi_pool = ctx.enter_context(tc.tile_pool(name="xi", bufs=2))
        kt_pool = ctx.enter_context(tc.tile_pool(name="kt", bufs=2))
        qt_pool = ctx.enter_context(tc.tile_pool(name="qt", bufs=2))
        kp_pool = ctx.enter_context(tc.tile_pool(name="kp", bufs=2))
        et_pool = ctx.enter_context(tc.tile_pool(name="et", bufs=ET_BUFS))
        s_pool = ctx.enter_context(tc.tile_pool(name="s", bufs=4))
        misc_pool = ctx.enter_context(tc.tile_pool(name="misc", bufs=4))
        res_pool = ctx.enter_context(tc.tile_pool(name="res", bufs=4))
        # PSUM: shared ring (scores/proj/transpose/denominator) + AV accum
        mm_psum = ctx.enter_context(tc.tile_pool(name="mmps", bufs=2, space="PSUM"))
        av_psum = ctx.enter_context(tc.tile_pool(name="avps", bufs=2, space="PSUM"))

        # ---- constants + all input DMAs, issued up front in use order ----
        cb_t = const.tile([128, 1024], BF16)
        cf_t = const.tile([128, 8], F32)
        warm = const.tile([128, 640], BF16)

        xl_ts, xi_ts = [], []
        for u in range(UPC):
            xl_ts.append(xl_pool.tile([128, NCI_L, 2048], BF16,
                                      name=f"xl_{u}", tag="xl"))
            xi_ts.append(xi_pool.tile([128, NCI_I, 2048], BF16,
                                      name=f"xi_{u}", tag="xi"))

        nc.sync.dma_start(cb_t[:], cb)
        nc.sync.dma_start(xl_ts[0][:, :, ds(0, 1024)], xl[0, 0])
        nc.sync.dma_start(cf_t[:], cf)
        nc.sync.dma_start(xl_ts[0][:, :, ds(1024, 1024)], xl[0, 1])
        nc.sync.dma_start(xi_ts[0][:, :, ds(0, 1024)], xi[0, 0])
        nc.sync.dma_start(xi_ts[0][:, :, ds(1024, 1024)], xi[0, 1])
        for u in range(1, UPC):
            for h in range(2):
                nc.sync.dma_start(xl_ts[u][:, :, ds(h * 1024, 1024)], xl[u, h])
            for h in range(2):
                nc.sync.dma_start(xi_ts[u][:, :, ds(h * 1024, 1024)], xi[u, h])

        # ---- PE warm-up: keep HAM busy while input DMAs land ----
        if K_WARM:
            nc.gpsimd.memset(warm[:], 0)
            warm_ps = mm_psum.tile([128, 512], F32, name="warm_ps", tag="mm")
            for _ in range(K_WARM):
                nc.tensor.matmul(warm_ps[:], warm[:, ds(512, 128)],
                                 warm[:, ds(0, 512)], start=True, stop=True)

        ident = cb_t[:, ds(CB_IDENT, 128)]
        invw1 = cb_t[:, ds(CB_INVW1, 128)]
        esc_ap = cf_t[:, ds(CF_ESC, 1)]
        scha_ap = cf_t[:, ds(CF_SCHA, 1)]
        schb_ap = cf_t[:, ds(CF_SCHB, 1)]

        kTb = [kt_pool.tile([128, CHUNK], BF16, name=f"kT_{u}", tag="kt")
               for u in range(UPC)]
        qTb = [qt_pool.tile([128, CHUNK], BF16, name=f"qT_{u}", tag="qt")
               for u in range(UPC)]
        kp = [kp_pool.tile([128, CHUNK], BF16, name=f"kp_{u}", tag="kp")
              for u in range(UPC)]

        def relu_store(dst, ps, bias_ap, eng):
            if eng == "A":
                nc.scalar.activation(dst, ps, mybir.ActivationFunctionType.Relu,
                                     bias=bias_ap)
            else:
                nc.vector.tensor_scalar(dst, ps, bias_ap, 0.0,
                                        op0=mybir.AluOpType.add,
                                        op1=mybir.AluOpType.max)

        def proj_k(u):
            """kTb[u] = relu(wlid.T @ xlid + b), bf16; then kp via PE transpose."""
            for half in range(2):
                psk = mm_psum.tile([128, QB], F32, name=f"psk_{u}_{half}",
                                   tag="mm")
                for b2 in range(2):
                    blk = half * 2 + b2
                    for ci in range(NCI_L):
                        nc.tensor.matmul(
                            psk[:, ts(b2, 512)],
                            cb_t[:, ds(CB_WLID + ci * 128, 128)],
                            xl_ts[u][:, ci, ds(blk * 512, 512)],
                            start=(ci == 0), stop=(ci == NCI_L - 1))
                relu_store(kTb[u][:, ts(half, QB)], psk[:],
                           cf_t[:, ds(CF_LID_B, 1)], RELU_ENG[u * 4 + half])
            for g in range(2):
                pt = mm_psum.tile([128, 1024], BF16, name=f"pt_{u}_{g}",
                                  tag="mm")
                for k8 in range(8):
                    nc.tensor.transpose(pt[:, ts(k8, 128)],
                                        kTb[u][:, ds(g * 1024 + k8 * 128, 128)],
                                        ident)
                nc.vector.tensor_copy(kp[u][:, ts(g, 1024)], pt[:])

        def proj_q(u):
            for half in range(2):
                psq = mm_psum.tile([128, QB], F32, name=f"psq_{u}_{half}",
                                   tag="mm")
                for b2 in range(2):
                    blk = half * 2 + b2
                    for ci in range(NCI_I):
                        nc.tensor.matmul(
                            psq[:, ts(b2, 512)],
                            cb_t[:, ds(CB_WIMG + ci * 128, 128)],
                            xi_ts[u][:, ci, ds(blk * 512, 512)],
                            start=(ci == 0), stop=(ci == NCI_I - 1))
                relu_store(qTb[u][:, ts(half, QB)], psq[:],
                           cf_t[:, ds(CF_IMG_B, 1)], RELU_ENG[u * 4 + 2 + half])

        def attn(u, qb):
            po = av_psum.tile([128, QB], F32, name=f"po_{u}_{qb}", tag="av")
            # ping-pong S so DVE adds never read+write the same tile
            Ss = [s_pool.tile([128, QB], BF16, name=f"S{t}_{u}_{qb}", tag="S")
                  for t in range(2)]
            ets = [None] * KSL

            def s_add(i):
                # S_{i} = S_{i-1} + et_i  (i >= 1); S_1 = et_0 + et_1
                src0 = ets[0][:] if i == 1 else Ss[i % 2][:]
                eng = nc.vector if ADD_ENG[i] == "v" else nc.gpsimd
                eng.tensor_add(Ss[(i + 1) % 2][:], src0, ets[i][:])

            for i in range(KSL + max(LOOKAHEAD, ADD_LAG + 1)):
                if i < KSL:
                    ps = mm_psum.tile([128, QB], F32,
                                      name=f"pss_{u}_{qb}_{i}", tag="mm")
                    for h in range(2):
                        nc.tensor.matmul(ps[:, ts(h, 512)],
                                         kTb[u][:, ds(i * 128, 128)],
                                         qTb[u][:, ds(qb * QB + h * 512, 512)],
                                         start=True, stop=True)
                    et = et_pool.tile([128, QB], BF16,
                                      name=f"et_{u}_{qb}_{i}", tag="et")
                    if EXP_ENG[i] == "A":
                        nc.scalar.activation(et[:], ps[:],
                                             mybir.ActivationFunctionType.Exp,
                                             scale=esc_ap)
                    else:
                        nc.vector.tensor_scalar(et[:].bitcast(I16), ps[:],
                                                scha_ap, schb_ap,
                                                op0=mybir.AluOpType.mult,
                                                op1=mybir.AluOpType.add)
                    ets[i] = et
                a = i - ADD_LAG
                if 1 <= a < KSL:
                    s_add(a)
                j = i - LOOKAHEAD
                if 0 <= j < KSL:
                    for h in range(2):
                        nc.tensor.matmul(po[:, ts(h, 512)],
                                         kp[u][:, ds(j * 128, 128)],
                                         ets[j][:, ts(h, 512)],
                                         start=(j == 0), stop=(j == KSL - 1))
            S = Ss[KSL % 2]
            # denominator: PE broadcast-sum (1/w1)^T @ S, then reciprocal
            pl = mm_psum.tile([128, QB], F32, name=f"pl_{u}_{qb}", tag="mm")
            for h in range(2):
                nc.tensor.matmul(pl[:, ts(h, 512)], invw1, S[:, ts(h, 512)],
                                 start=True, stop=True)
            linv = misc_pool.tile([128, QB], F32, name=f"linv_{u}_{qb}",
                                  tag="linv")
            nc.vector.reciprocal_approx_fast(linv[:], pl[:])
            # res = po * (w1/L) + qTb
            tmp = misc_pool.tile([128, QB], F32, name=f"tmp_{u}_{qb}",
                                 tag="tmp")
            nc.vector.tensor_mul(tmp[:], po[:], linv[:])
            res = res_pool.tile([128, QB], F32, name=f"res_{u}_{qb}",
                                tag="res")
            nc.vector.tensor_add(res[:], tmp[:], qTb[u][:, ts(qb, QB)])
            nc.sync.dma_start(y[u][:, ts(qb, QB)], res[:])

        # unit 0 proj, then attention with unit-1 proj emitted between
        # q-blocks so its matmuls/relus backfill the exp-paced pipeline
        proj_k(0)
        proj_q(0)
        attn(0, 0)
        proj_k(1)
        proj_q(1)
        attn(0, 1)
        attn(1, 0)
        attn(1, 1)

    nc.compile()
    return nc


def _shard_inputs(inputs):
    """Build the 8 per-core input maps from the full input dict."""
    bf = mybir.dt.np(BF16)
    mw = np.asarray(inputs["modality_weights"], np.float64)
    e = np.exp(mw - mw.max())
    w = (e / e.sum()).astype(np.float64)
    w0, w1 = float(w[0]), float(w[1])

    def bn_fold(gamma, beta, mean, var, mul):
        g = np.asarray(gamma, np.float64)
        b = np.asarray(beta, np.float64)
        m = np.asarray(mean, np.float64)
        v = np.asarray(var, np.float64)
        scale = g / np.sqrt(v + EPS) * mul
        bias = (b - m * g / np.sqrt(v + EPS)) * mul
        return scale, bias

    i_s, i_b = bn_fold(inputs["image_gamma"], inputs["image_beta"],
                       inputs["image_mean"], inputs["image_var"], w0)
    l_s, l_b = bn_fold(inputs["lidar_gamma"], inputs["lidar_beta"],
                       inputs["lidar_mean"], inputs["lidar_var"], 1.0)

    # weight slices, pre-transposed for lhsT ([cin_slice, cout]), BN scale
    # folded in, bf16
    wi = (np.asarray(inputs["image_w"], np.float64).T * i_s[None, :])
    wl = (np.asarray(inputs["lidar_w"], np.float64).T * l_s[None, :])
    wi = wi.astype(np.float32).astype(bf).reshape(NCI_I, 128, CO)
    wl = wl.astype(np.float32).astype(bf).reshape(NCI_L, 128, CO)

    cb = np.zeros((128, 1024), bf)
    for ci in range(NCI_I):
        cb[:, CB_WIMG + ci * 128: CB_WIMG + (ci + 1) * 128] = wi[ci]
    for ci in range(NCI_L):
        cb[:, CB_WLID + ci * 128: CB_WLID + (ci + 1) * 128] = wl[ci]
    cb[:, CB_IDENT:CB_IDENT + 128] = np.eye(128, dtype=bf)
    cb[:, CB_INVW1:CB_INVW1 + 128] = np.full((128, 128), 1.0 / w1, bf)

    escv = 1.0 / (w0 * math.sqrt(CO))
    cfv = np.zeros((128, 8), np.float32)
    cfv[:, CF_IMG_B] = i_b.astype(np.float32)
    cfv[:, CF_LID_B] = l_b.astype(np.float32)
    cfv[:, CF_ESC] = escv
    cfv[:, CF_SCHA] = escv * SCH_A
    cfv[:, CF_SCHB] = 127.0 * 128.0 - SCH_C

    # features -> (B, C, nchunks, 2048) bf16
    img = np.asarray(inputs["image_features"], np.float32).astype(bf) \
        .reshape(B, NCI_I, 128, NCH, CHUNK)
    lid = np.asarray(inputs["lidar_features"], np.float32).astype(bf) \
        .reshape(B, NCI_L, 128, NCH, CHUNK)

    in_maps = []
    for core in range(NCORES):
        ximg = np.empty((UPC, 2, 128, NCI_I, 1024), bf)
        xlid = np.empty((UPC, 2, 128, NCI_L, 1024), bf)
        for ul in range(UPC):
            un = core * UPC + ul
            b, c = un // NCH, un % NCH
            for h in range(2):
                # [ci, 128, 1024] -> [128, ci, 1024]
                ximg[ul, h] = img[b, :, :, c, h * 1024:(h + 1) * 1024] \
                    .transpose(1, 0, 2)
                xlid[ul, h] = lid[b, :, :, c, h * 1024:(h + 1) * 1024] \
                    .transpose(1, 0, 2)
        in_maps.append({"cb": cb, "cf": cfv, "xl": xlid, "xi": ximg})
    return in_maps


def kernel(**inputs) -> np.ndarray:
    global _PROGRAM, LAST_RESULTS
    if _PROGRAM is None:
        _PROGRAM = _build_program()
    nc = _PROGRAM

    in_maps = _shard_inputs(inputs)
    trace = os.environ.get("BASS_KERNEL_TRACE", "0") == "1"
    tmpdir = os.environ.get("BASS_KERNEL_TRACE_DIR") or None
    if tmpdir:
        os.makedirs(tmpdir, exist_ok=True)
    results = run_bass_kernel_spmd(nc, in_maps, core_ids=list(range(NCORES)),
                                   trace=trace, tmpdir=tmpdir)
    LAST_RESULTS = results

    out = np.empty((B, CO, H, W), np.float32)
    outv = out.reshape(B, CO, NCH, CHUNK)
    for core in range(NCORES):
        yc = results.results[core]["y"]
        for ul in range(UPC):
            un = core * UPC + ul
            b, c = un // NCH, un % NCH
            outv[b, :, c, :] = yc[ul]
    return out


if __name__ == "__main__":
    rng = np.random.default_rng(0)
    inputs = {
        "lidar_features": rng.standard_normal((B, CL, H, W), np.float32),
        "image_features": rng.standard_normal((B, CI, H, W), np.float32),
        "lidar_w": rng.standard_normal((CO, CL), np.float32) * np.sqrt(2.0 / CO),
        "lidar_gamma": np.ones(CO, np.float32),
        "lidar_beta": np.zeros(CO, np.float32),
        "lidar_mean": rng.standard_normal(CO).astype(np.float32) * 0.1,
        "lidar_var": rng.uniform(0.5, 1.5, CO).astype(np.float32),
        "image_w": rng.standard_normal((CO, CI), np.float32) * np.sqrt(2.0 / CO),
        "image_gamma": np.ones(CO, np.float32),
        "image_beta": np.zeros(CO, np.float32),
        "image_mean": rng.standard_normal(CO).astype(np.float32) * 0.1,
        "image_var": rng.uniform(0.5, 1.5, CO).astype(np.float32),
        "modality_weights": np.ones(2, np.float32),
    }
    out = kernel(**inputs)
    print("kernel out:", out.shape, out.dtype, float(np.abs(out).mean()))


# revision 7
# speedup vs baseline: 1.3255x; 1.0195x over previous
"""CrossAttentionFusion kernel for Trainium2 (8 NeuronCores, Bass/Tile).

Computation (matches the reference nn.Module):
  image_proj = relu(BN(1x1conv(image_features, image_w)))   # (B,128,H,W)
  lidar_proj = relu(BN(1x1conv(lidar_features, lidar_w)))   # (B,128,H,W)
  per (batch, 2048-pixel chunk): q = image_proj, k = v = lidar_proj
  attn_out = softmax(q k^T / sqrt(128)) @ k
  out = w0 * image_proj + w1 * attn_out,  w = softmax(modality_weights)

Sharding: the 16 independent (batch, chunk) attention problems are
distributed 2-per-core across 8 cores; each core computes the projections
for its own pixels.  Host gathers the 8 outputs.

v2 design (all-bf16 data path):
  - Inputs and conv weights are cast to bf16 on the host (BN scale folded
    into the weights, w0 folded into the image weights/bias); f32 PSUM
    accumulation keeps the projections accurate; ACT applies bias+relu and
    writes qTb/kTb directly as bf16 (no separate downcast pass).
  - Scores: sT[k,q] = kTb_slice.T @ qTb (bf16 in, f32 psum), exp split
    between ACT (true exp -> bf16) and DVE (Schraudolph int16 bit-trick).
  - Softmax denominator: S = sum_i et_i on DVE (bf16), partition-reduced
    and broadcast by one (1/w1)-matrix matmul, reciprocal on DVE gives
    linv = w1/L, so the output is just po*linv + qTb (2 DVE ops).
  - AV: kp (pixel-major bf16 via PE transposes) @ et, f32 psum accum.
  - DMAs: 2 packed const DMAs + 2 per (unit, modality) issued up front in
    consumption order; dummy warm-up matmuls keep the PE HAM warm through
    the DMA head; per-q-block output DMAs shrink the tail.
"""

import math
import os
import sys
from contextlib import ExitStack

import numpy as np

sys.path.insert(0, "/opt/trn_rl_repo")

import concourse.bass as bass  # noqa: E402
import concourse.tile as tile  # noqa: E402
from concourse import bacc, mybir  # noqa: E402
from concourse.bass import ds, ts  # noqa: E402
from concourse.bass_utils import run_bass_kernel_spmd  # noqa: E402

F32 = mybir.dt.float32
BF16 = mybir.dt.bfloat16
I16 = mybir.dt.int16

B, CL, CI, CO = 2, 256, 512, 128
H = W = 128
P = H * W                    # 16384 pixels per batch
CHUNK = 2048                 # attention chunk (pixels)
NCH = P // CHUNK             # 8 chunks per batch
NCORES = 8
UPC = (B * NCH) // NCORES    # units (b,chunk) per core = 2
EPS = 1e-5
QB = 1024                    # q-block width (2 matmul halves of 512)
NQB = CHUNK // QB            # 2
KSL = CHUNK // 128           # 16 k-pixel slices per chunk
NCI_I = CI // 128            # 4 contraction slices for image proj
NCI_L = CL // 128            # 2 for lidar proj

# exp engine per k-slice index: A=ACT true exp, D=DVE Schraudolph.
EXP_ENG = os.environ.get("K_EXP_ENG", "AADAAAAAADAAAAAA")
# S-add engine per add index 1..15: v=DVE, g=GPSIMD
ADD_ENG = os.environ.get("K_ADD_ENG", "vvvvvvvvvvvvvvvv")
ADD_LAG = int(os.environ.get("K_ADD_LAG", "3"))
LOOKAHEAD = int(os.environ.get("K_LOOKAHEAD", "2"))
K_WARM = int(os.environ.get("K_WARM", "8"))
# proj relu engine, 8 chars: u0[kh0,kh1,qh0,qh1], u1[...]  A=ACT, D=DVE
RELU_ENG = os.environ.get("K_RELU_ENG", "AAAAAAAA")
ET_BUFS = int(os.environ.get("K_ET_BUFS", "18"))
MM_BUFS = int(os.environ.get("K_MM_BUFS", "2"))
AV_BUFS = int(os.environ.get("K_AV_BUFS", "2"))

# Schraudolph constants for bf16-bits-in-int16 exp approximation
# (DVE float->int conversion truncates; C tuned for that):
#   exp(x) ~= bitcast_bf16(int16(x * 128/ln2 + (127*128 - C)))
SCH_A = 128.0 / math.log(2.0)
SCH_C = 5.0

# cf (f32 const) column indices
CF_IMG_B, CF_LID_B, CF_ESC, CF_SCHA, CF_SCHB = 0, 1, 2, 3, 4
# cb (bf16 const) column offsets
CB_WIMG, CB_WLID, CB_IDENT, CB_INVW1 = 0, 512, 768, 896

_PROGRAM = None              # compiled Bass program, built once per process
LAST_RESULTS = None          # BassKernelResults of the last kernel() call


def _build_program():
    nc = bacc.Bacc("TRN2", target_bir_lowering=False, debug=False,
                   num_devices=NCORES)

    cb = nc.dram_tensor("cb", [128, 1024], BF16, kind="ExternalInput").ap()
    cf = nc.dram_tensor("cf", [128, 8], F32, kind="ExternalInput").ap()
    # per-(unit,pixel-half) inputs, ci-major within the SBUF row
    xl = nc.dram_tensor("xl", [UPC, 2, 128, NCI_L, 1024], BF16,
                        kind="ExternalInput").ap()
    xi = nc.dram_tensor("xi", [UPC, 2, 128, NCI_I, 1024], BF16,
                        kind="ExternalInput").ap()
    y = nc.dram_tensor("y", [UPC, CO, CHUNK], F32, kind="ExternalOutput").ap()

    with tile.TileContext(nc) as tc, ExitStack() as ctx:
        const = ctx.enter_context(tc.tile_pool(name="const", bufs=1))
        xl_pool = ctx.enter_context(tc.tile_pool(name="xl", bufs=2))
        xi_pool = ctx.enter_context(tc.tile_pool(name="xi", bufs=2))
        kt_pool = ctx.enter_context(tc.tile_pool(name="kt", bufs=2))
        qt_pool = ctx.enter_context(tc.tile_pool(name="qt", bufs=2))
        kp_pool = ctx.enter_context(tc.tile_pool(name="kp", bufs=2))
        et_pool = ctx.enter_context(tc.tile_pool(name="et", bufs=ET_BUFS))
        s_pool = ctx.enter_context(tc.tile_pool(name="s", bufs=4))
        misc_pool = ctx.enter_context(tc.tile_pool(name="misc", bufs=4))
        res_pool = ctx.enter_context(tc.tile_pool(name="res", bufs=4))
        # PSUM: shared ring (scores/proj/transpose/denominator) + AV accum
        mm_psum = ctx.enter_context(tc.tile_pool(name="mmps", bufs=MM_BUFS, space="PSUM"))
        av_psum = ctx.enter_context(tc.tile_pool(name="avps", bufs=AV_BUFS, space="PSUM"))

        # ---- constants + all input DMAs, issued up front in use order ----
        cb_t = const.tile([128, 1024], BF16)
        cf_t = const.tile([128, 8], F32)
        warm = const.tile([128, 640], BF16)

        xl_ts, xi_ts = [], []
        for u in range(UPC):
            xl_ts.append(xl_pool.tile([128, NCI_L, 2048], BF16,
                                      name=f"xl_{u}", tag="xl"))
            xi_ts.append(xi_pool.tile([128, NCI_I, 2048], BF16,
                                      name=f"xi_{u}", tag="xi"))

        nc.sync.dma_start(cb_t[:], cb)
        nc.sync.dma_start(xl_ts[0][:, :, ds(0, 1024)], xl[0, 0])
        nc.sync.dma_start(cf_t[:], cf)
        nc.sync.dma_start(xl_ts[0][:, :, ds(1024, 1024)], xl[0, 1])
        nc.sync.dma_start(xi_ts[0][:, :, ds(0, 1024)], xi[0, 0])
        nc.sync.dma_start(xi_ts[0][:, :, ds(1024, 1024)], xi[0, 1])
        for u in range(1, UPC):
            for h in range(2):
                nc.sync.dma_start(xl_ts[u][:, :, ds(h * 1024, 1024)], xl[u, h])
            for h in range(2):
                nc.sync.dma_start(xi_ts[u][:, :, ds(h * 1024, 1024)], xi[u, h])

        # ---- PE warm-up: keep HAM busy while input DMAs land ----
        if K_WARM:
            nc.gpsimd.memset(warm[:], 0)
            warm_ps = mm_psum.tile([128, 512], F32, name="warm_ps", tag="mm")
            for _ in range(K_WARM):
                nc.tensor.matmul(warm_ps[:], warm[:, ds(512, 128)],
                                 warm[:, ds(0, 512)], start=True, stop=True)

        ident = cb_t[:, ds(CB_IDENT, 128)]
        invw1 = cb_t[:, ds(CB_INVW1, 128)]
        esc_ap = cf_t[:, ds(CF_ESC, 1)]
        scha_ap = cf_t[:, ds(CF_SCHA, 1)]
        schb_ap = cf_t[:, ds(CF_SCHB, 1)]

        kTb = [kt_pool.tile([128, CHUNK], BF16, name=f"kT_{u}", tag="kt")
               for u in range(UPC)]
        qTb = [qt_pool.tile([128, CHUNK], BF16, name=f"qT_{u}", tag="qt")
               for u in range(UPC)]
        kp = [kp_pool.tile([128, CHUNK], BF16, name=f"kp_{u}", tag="kp")
              for u in range(UPC)]

        def relu_store(dst, ps, bias_ap, eng):
            if eng == "A":
                nc.scalar.activation(dst, ps, mybir.ActivationFunctionType.Relu,
                                     bias=bias_ap)
            else:
                nc.vector.tensor_scalar(dst, ps, bias_ap, 0.0,
                                        op0=mybir.AluOpType.add,
                                        op1=mybir.AluOpType.max)

        def proj_k(u):
            """kTb[u] = relu(wlid.T @ xlid + b), bf16; then kp via PE transpose."""
            for half in range(2):
                psk = mm_psum.tile([128, QB], F32, name=f"psk_{u}_{half}",
                                   tag="mm")
                for b2 in range(2):
                    blk = half * 2 + b2
                    for ci in range(NCI_L):
                        nc.tensor.matmul(
                            psk[:, ts(b2, 512)],
                            cb_t[:, ds(CB_WLID + ci * 128, 128)],
                            xl_ts[u][:, ci, ds(blk * 512, 512)],
                            start=(ci == 0), stop=(ci == NCI_L - 1))
                relu_store(kTb[u][:, ts(half, QB)], psk[:],
                           cf_t[:, ds(CF_LID_B, 1)], RELU_ENG[u * 4 + half])
            for g in range(2):
                pt = mm_psum.tile([128, 1024], BF16, name=f"pt_{u}_{g}",
                                  tag="mm")
                for k8 in range(8):
                    nc.tensor.transpose(pt[:, ts(k8, 128)],
                                        kTb[u][:, ds(g * 1024 + k8 * 128, 128)],
                                        ident)
                nc.vector.tensor_copy(kp[u][:, ts(g, 1024)], pt[:])

        def proj_q(u):
            for half in range(2):
                psq = mm_psum.tile([128, QB], F32, name=f"psq_{u}_{half}",
                                   tag="mm")
                for b2 in range(2):
                    blk = half * 2 + b2
                    for ci in range(NCI_I):
                        nc.tensor.matmul(
                            psq[:, ts(b2, 512)],
                            cb_t[:, ds(CB_WIMG + ci * 128, 128)],
                            xi_ts[u][:, ci, ds(blk * 512, 512)],
                            start=(ci == 0), stop=(ci == NCI_I - 1))
                relu_store(qTb[u][:, ts(half, QB)], psq[:],
                           cf_t[:, ds(CF_IMG_B, 1)], RELU_ENG[u * 4 + 2 + half])

        def attn(u, qb, drain_fast=False):
            lag = 1 if drain_fast else ADD_LAG
            po = av_psum.tile([128, QB], F32, name=f"po_{u}_{qb}", tag="av")
            # ping-pong S so DVE adds never read+write the same tile
            Ss = [s_pool.tile([128, QB], BF16, name=f"S{t}_{u}_{qb}", tag="S")
                  for t in range(2)]
            ets = [None] * KSL

            def s_add(i):
                # S_{i} = S_{i-1} + et_i  (i >= 1); S_1 = et_0 + et_1
                src0 = ets[0][:] if i == 1 else Ss[i % 2][:]
                eng = nc.vector if ADD_ENG[i] == "v" else nc.gpsimd
                eng.tensor_add(Ss[(i + 1) % 2][:], src0, ets[i][:])

            for i in range(KSL + max(LOOKAHEAD, lag + 1)):
                if i < KSL:
                    ps = mm_psum.tile([128, QB], F32,
                                      name=f"pss_{u}_{qb}_{i}", tag="mm")
                    for h in range(2):
                        nc.tensor.matmul(ps[:, ts(h, 512)],
                                         kTb[u][:, ds(i * 128, 128)],
                                         qTb[u][:, ds(qb * QB + h * 512, 512)],
                                         start=True, stop=True)
                    et = et_pool.tile([128, QB], BF16,
                                      name=f"et_{u}_{qb}_{i}", tag="et")
                    if EXP_ENG[i] == "A":
                        nc.scalar.activation(et[:], ps[:],
                                             mybir.ActivationFunctionType.Exp,
                                             scale=esc_ap)
                    else:
                        nc.vector.tensor_scalar(et[:].bitcast(I16), ps[:],
                                                scha_ap, schb_ap,
                                                op0=mybir.AluOpType.mult,
                                                op1=mybir.AluOpType.add)
                    ets[i] = et
                a = i - lag
                if 1 <= a < KSL:
                    s_add(a)
                j = i - LOOKAHEAD
                if 0 <= j < KSL:
                    for h in range(2):
                        nc.tensor.matmul(po[:, ts(h, 512)],
                                         kp[u][:, ds(j * 128, 128)],
                                         ets[j][:, ts(h, 512)],
                                         start=(j == 0), stop=(j == KSL - 1))
            S = Ss[KSL % 2]
            # denominator: PE broadcast-sum (1/w1)^T @ S, then reciprocal
            pl = mm_psum.tile([128, QB], F32, name=f"pl_{u}_{qb}", tag="mm")
            for h in range(2):
                nc.tensor.matmul(pl[:, ts(h, 512)], invw1, S[:, ts(h, 512)],
                                 start=True, stop=True)
            # res = po * (w1/L) + qTb; fast drain pipelines 512-wide halves
            # straight into the output DMA to shrink the kernel tail
            nhalf = 2 if drain_fast else 1
            wd = QB // nhalf
            for h in range(nhalf):
                linv = misc_pool.tile([128, wd], F32, name=f"linv_{u}_{qb}_{h}",
                                      tag="linv")
                nc.vector.reciprocal_approx_fast(linv[:], pl[:, ts(h, wd)])
                tmp = misc_pool.tile([128, wd], F32, name=f"tmp_{u}_{qb}_{h}",
                                     tag="tmp")
                nc.vector.tensor_mul(tmp[:], po[:, ts(h, wd)], linv[:])
                res = res_pool.tile([128, wd], F32, name=f"res_{u}_{qb}_{h}",
                                    tag="res")
                nc.vector.tensor_add(res[:], tmp[:],
                                     qTb[u][:, ds(qb * QB + h * wd, wd)])
                nc.sync.dma_start(y[u][:, ds(qb * QB + h * wd, wd)], res[:])

        # unit 0 proj, then attention with unit-1 proj emitted between
        # q-blocks so its matmuls/relus backfill the exp-paced pipeline
        proj_k(0)
        proj_q(0)
        attn(0, 0)
        proj_k(1)
        proj_q(1)
        attn(0, 1)
        attn(1, 0)
        attn(1, 1, drain_fast=True)

    nc.compile()
    return nc


def _shard_inputs(inputs):
    """Build the 8 per-core input maps from the full input dict."""
    bf = mybir.dt.np(BF16)
    mw = np.asarray(inputs["modality_weights"], np.float64)
    e = np.exp(mw - mw.max())
    w = (e / e.sum()).astype(np.float64)
    w0, w1 = float(w[0]), float(w[1])

    def bn_fold(gamma, beta, mean, var, mul):
        g = np.asarray(gamma, np.float64)
        b = np.asarray(beta, np.float64)
        m = np.asarray(mean, np.float64)
        v = np.asarray(var, np.float64)
        scale = g / np.sqrt(v + EPS) * mul
        bias = (b - m * g / np.sqrt(v + EPS)) * mul
        return scale, bias

    i_s, i_b = bn_fold(inputs["image_gamma"], inputs["image_beta"],
                       inputs["image_mean"], inputs["image_var"], w0)
    l_s, l_b = bn_fold(inputs["lidar_gamma"], inputs["lidar_beta"],
                       inputs["lidar_mean"], inputs["lidar_var"], 1.0)

    # weight slices, pre-transposed for lhsT ([cin_slice, cout]), BN scale
    # folded in, bf16
    wi = (np.asarray(inputs["image_w"], np.float64).T * i_s[None, :])
    wl = (np.asarray(inputs["lidar_w"], np.float64).T * l_s[None, :])
    wi = wi.astype(np.float32).astype(bf).reshape(NCI_I, 128, CO)
    wl = wl.astype(np.float32).astype(bf).reshape(NCI_L, 128, CO)

    cb = np.zeros((128, 1024), bf)
    for ci in range(NCI_I):
        cb[:, CB_WIMG + ci * 128: CB_WIMG + (ci + 1) * 128] = wi[ci]
    for ci in range(NCI_L):
        cb[:, CB_WLID + ci * 128: CB_WLID + (ci + 1) * 128] = wl[ci]
    cb[:, CB_IDENT:CB_IDENT + 128] = np.eye(128, dtype=bf)
    cb[:, CB_INVW1:CB_INVW1 + 128] = np.full((128, 128), 1.0 / w1, bf)

    escv = 1.0 / (w0 * math.sqrt(CO))
    cfv = np.zeros((128, 8), np.float32)
    cfv[:, CF_IMG_B] = i_b.astype(np.float32)
    cfv[:, CF_LID_B] = l_b.astype(np.float32)
    cfv[:, CF_ESC] = escv
    cfv[:, CF_SCHA] = escv * SCH_A
    cfv[:, CF_SCHB] = 127.0 * 128.0 - SCH_C

    # features -> (B, C, nchunks, 2048) bf16
    img = np.asarray(inputs["image_features"], np.float32).astype(bf) \
        .reshape(B, NCI_I, 128, NCH, CHUNK)
    lid = np.asarray(inputs["lidar_features"], np.float32).astype(bf) \
        .reshape(B, NCI_L, 128, NCH, CHUNK)

    in_maps = []
    for core in range(NCORES):
        ximg = np.empty((UPC, 2, 128, NCI_I, 1024), bf)
        xlid = np.empty((UPC, 2, 128, NCI_L, 1024), bf)
        for ul in range(UPC):
            un = core * UPC + ul
            b, c = un // NCH, un % NCH
            for h in range(2):
                # [ci, 128, 1024] -> [128, ci, 1024]
                ximg[ul, h] = img[b, :, :, c, h * 1024:(h + 1) * 1024] \
                    .transpose(1, 0, 2)
                xlid[ul, h] = lid[b, :, :, c, h * 1024:(h + 1) * 1024] \
                    .transpose(1, 0, 2)
        in_maps.append({"cb": cb, "cf": cfv, "xl": xlid, "xi": ximg})
    return in_maps


def kernel(**inputs) -> np.ndarray:
    global _PROGRAM, LAST_RESULTS
    if _PROGRAM is None:
        _PROGRAM = _build_program()
    nc = _PROGRAM

    in_maps = _shard_inputs(inputs)
    trace = os.environ.get("BASS_KERNEL_TRACE", "0") == "1"
    tmpdir = os.environ.get("BASS_KERNEL_TRACE_DIR") or None
    if tmpdir:
        os.makedirs(tmpdir, exist_ok=True)
    results = run_bass_kernel_spmd(nc, in_maps, core_ids=list(range(NCORES)),
                                   trace=trace, tmpdir=tmpdir)
    LAST_RESULTS = results

    out = np.empty((B, CO, H, W), np.float32)
    outv = out.reshape(B, CO, NCH, CHUNK)
    for core in range(NCORES):
        yc = results.results[core]["y"]
        for ul in range(UPC):
            un = core * UPC + ul
            b, c = un // NCH, un % NCH
            outv[b, :, c, :] = yc[ul]
    return out


if __name__ == "__main__":
    rng = np.random.default_rng(0)
    inputs = {
        "lidar_features": rng.standard_normal((B, CL, H, W), np.float32),
        "image_features": rng.standard_normal((B, CI, H, W), np.float32),
        "lidar_w": rng.standard_normal((CO, CL), np.float32) * np.sqrt(2.0 / CO),
        "lidar_gamma": np.ones(CO, np.float32),
        "lidar_beta": np.zeros(CO, np.float32),
        "lidar_mean": rng.standard_normal(CO).astype(np.float32) * 0.1,
        "lidar_var": rng.uniform(0.5, 1.5, CO).astype(np.float32),
        "image_w": rng.standard_normal((CO, CI), np.float32) * np.sqrt(2.0 / CO),
        "image_gamma": np.ones(CO, np.float32),
        "image_beta": np.zeros(CO, np.float32),
        "image_mean": rng.standard_normal(CO).astype(np.float32) * 0.1,
        "image_var": rng.uniform(0.5, 1.5, CO).astype(np.float32),
        "modality_weights": np.ones(2, np.float32),
    }
    out = kernel(**inputs)
    print("kernel out:", out.shape, out.dtype, float(np.abs(out).mean()))


# revision 11
# speedup vs baseline: 1.3585x; 1.0249x over previous
"""CrossAttentionFusion kernel for Trainium2 (8 NeuronCores, Bass/Tile).

Computation (matches the reference nn.Module):
  image_proj = relu(BN(1x1conv(image_features, image_w)))   # (B,128,H,W)
  lidar_proj = relu(BN(1x1conv(lidar_features, lidar_w)))   # (B,128,H,W)
  per (batch, 2048-pixel chunk): q = image_proj, k = v = lidar_proj
  attn_out = softmax(q k^T / sqrt(128)) @ k
  out = w0 * image_proj + w1 * attn_out,  w = softmax(modality_weights)

Sharding: the 16 independent (batch, chunk) attention problems are
distributed 2-per-core across 8 cores; each core computes the projections
for its own pixels.  Host gathers the 8 outputs.

v2 design (all-bf16 data path):
  - Inputs and conv weights are cast to bf16 on the host (BN scale folded
    into the weights, w0 folded into the image weights/bias); f32 PSUM
    accumulation keeps the projections accurate; ACT applies bias+relu and
    writes qTb/kTb directly as bf16 (no separate downcast pass).
  - Scores: sT[k,q] = kTb_slice.T @ qTb (bf16 in, f32 psum), exp split
    between ACT (true exp -> bf16) and DVE (Schraudolph int16 bit-trick).
  - Softmax denominator: S = sum_i et_i on DVE (bf16), partition-reduced
    and broadcast by one (1/w1)-matrix matmul, reciprocal on DVE gives
    linv = w1/L, so the output is just po*linv + qTb (2 DVE ops).
  - AV: kp (pixel-major bf16 via PE transposes) @ et, f32 psum accum.
  - DMAs: 2 packed const DMAs + 2 per (unit, modality) issued up front in
    consumption order; dummy warm-up matmuls keep the PE HAM warm through
    the DMA head; per-q-block output DMAs shrink the tail.
"""

import math
import os
import sys
from contextlib import ExitStack

import numpy as np

sys.path.insert(0, "/opt/trn_rl_repo")

import concourse.bass as bass  # noqa: E402
import concourse.tile as tile  # noqa: E402
from concourse import bacc, mybir  # noqa: E402
from concourse.bass import ds, ts  # noqa: E402
from concourse.bass_utils import run_bass_kernel_spmd  # noqa: E402

F32 = mybir.dt.float32
BF16 = mybir.dt.bfloat16
I16 = mybir.dt.int16

B, CL, CI, CO = 2, 256, 512, 128
H = W = 128
P = H * W                    # 16384 pixels per batch
CHUNK = 2048                 # attention chunk (pixels)
NCH = P // CHUNK             # 8 chunks per batch
NCORES = 8
UPC = (B * NCH) // NCORES    # units (b,chunk) per core = 2
EPS = 1e-5
QB = 1024                    # q-block width (2 matmul halves of 512)
NQB = CHUNK // QB            # 2
KSL = CHUNK // 128           # 16 k-pixel slices per chunk
NCI_I = CI // 128            # 4 contraction slices for image proj
NCI_L = CL // 128            # 2 for lidar proj

# exp engine per k-slice index: A=ACT true exp, D=DVE Schraudolph.
EXP_ENG = os.environ.get("K_EXP_ENG", "AADAAAAAADAAAAAA")
# S-add engine per add index 1..15: v=DVE, g=GPSIMD
ADD_ENG = os.environ.get("K_ADD_ENG", "vvvvvvvvvvvvvvvv")
ADD_LAG = int(os.environ.get("K_ADD_LAG", "3"))
LOOKAHEAD = int(os.environ.get("K_LOOKAHEAD", "3"))
K_WARM = int(os.environ.get("K_WARM", "8"))
# proj relu engine, 8 chars: u0[kh0,kh1,qh0,qh1], u1[...]  A=ACT, D=DVE
RELU_ENG = os.environ.get("K_RELU_ENG", "AAAAAAAA")
ET_BUFS = int(os.environ.get("K_ET_BUFS", "18"))
MM_BUFS = int(os.environ.get("K_MM_BUFS", "3"))
AV_BUFS = int(os.environ.get("K_AV_BUFS", "1"))

# Schraudolph constants for bf16-bits-in-int16 exp approximation
# (DVE float->int conversion truncates; C tuned for that):
#   exp(x) ~= bitcast_bf16(int16(x * 128/ln2 + (127*128 - C)))
SCH_A = 128.0 / math.log(2.0)
SCH_C = 5.0

# cf (f32 const) column indices
CF_IMG_B, CF_LID_B, CF_ESC, CF_SCHA, CF_SCHB = 0, 1, 2, 3, 4
# cb (bf16 const) column offsets
CB_WIMG, CB_WLID, CB_IDENT, CB_INVW1 = 0, 512, 768, 896

_PROGRAM = None              # compiled Bass program, built once per process
LAST_RESULTS = None          # BassKernelResults of the last kernel() call


def _build_program():
    nc = bacc.Bacc("TRN2", target_bir_lowering=False, debug=False,
                   num_devices=NCORES)

    cb = nc.dram_tensor("cb", [128, 1024], BF16, kind="ExternalInput").ap()
    cf = nc.dram_tensor("cf", [128, 8], F32, kind="ExternalInput").ap()
    # per-(unit,pixel-half) inputs, ci-major within the SBUF row
    xl = nc.dram_tensor("xl", [UPC, 2, 128, NCI_L, 1024], BF16,
                        kind="ExternalInput").ap()
    xi = nc.dram_tensor("xi", [UPC, 2, 128, NCI_I, 1024], BF16,
                        kind="ExternalInput").ap()
    y = nc.dram_tensor("y", [UPC, CO, CHUNK], F32, kind="ExternalOutput").ap()

    with tile.TileContext(nc) as tc, ExitStack() as ctx:
        const = ctx.enter_context(tc.tile_pool(name="const", bufs=1))
        xl_pool = ctx.enter_context(tc.tile_pool(name="xl", bufs=2))
        xi_pool = ctx.enter_context(tc.tile_pool(name="xi", bufs=2))
        kt_pool = ctx.enter_context(tc.tile_pool(name="kt", bufs=2))
        qt_pool = ctx.enter_context(tc.tile_pool(name="qt", bufs=2))
        kp_pool = ctx.enter_context(tc.tile_pool(name="kp", bufs=2))
        et_pool = ctx.enter_context(tc.tile_pool(name="et", bufs=ET_BUFS))
        s_pool = ctx.enter_context(tc.tile_pool(name="s", bufs=4))
        misc_pool = ctx.enter_context(tc.tile_pool(name="misc", bufs=4))
        res_pool = ctx.enter_context(tc.tile_pool(name="res", bufs=4))
        # PSUM: shared ring (scores/proj/transpose/denominator) + AV accum
        mm_psum = ctx.enter_context(tc.tile_pool(name="mmps", bufs=MM_BUFS, space="PSUM"))
        av_psum = ctx.enter_context(tc.tile_pool(name="avps", bufs=AV_BUFS, space="PSUM"))

        # ---- constants + all input DMAs, issued up front in use order ----
        cb_t = const.tile([128, 1024], BF16)
        cf_t = const.tile([128, 8], F32)
        warm = const.tile([128, 640], BF16)

        xl_ts, xi_ts = [], []
        for u in range(UPC):
            xl_ts.append(xl_pool.tile([128, NCI_L, 2048], BF16,
                                      name=f"xl_{u}", tag="xl"))
            xi_ts.append(xi_pool.tile([128, NCI_I, 2048], BF16,
                                      name=f"xi_{u}", tag="xi"))

        nc.sync.dma_start(cb_t[:], cb)
        nc.sync.dma_start(xl_ts[0][:, :, ds(0, 1024)], xl[0, 0])
        nc.sync.dma_start(cf_t[:], cf)
        nc.sync.dma_start(xl_ts[0][:, :, ds(1024, 1024)], xl[0, 1])
        nc.sync.dma_start(xi_ts[0][:, :, ds(0, 1024)], xi[0, 0])
        nc.sync.dma_start(xi_ts[0][:, :, ds(1024, 1024)], xi[0, 1])
        for u in range(1, UPC):
            for h in range(2):
                nc.sync.dma_start(xl_ts[u][:, :, ds(h * 1024, 1024)], xl[u, h])
            for h in range(2):
                nc.sync.dma_start(xi_ts[u][:, :, ds(h * 1024, 1024)], xi[u, h])

        # ---- PE warm-up: keep HAM busy while input DMAs land ----
        if K_WARM:
            nc.gpsimd.memset(warm[:], 0)
            warm_ps = mm_psum.tile([128, 512], F32, name="warm_ps", tag="mm")
            for _ in range(K_WARM):
                nc.tensor.matmul(warm_ps[:], warm[:, ds(512, 128)],
                                 warm[:, ds(0, 512)], start=True, stop=True)

        ident = cb_t[:, ds(CB_IDENT, 128)]
        invw1 = cb_t[:, ds(CB_INVW1, 128)]
        esc_ap = cf_t[:, ds(CF_ESC, 1)]
        scha_ap = cf_t[:, ds(CF_SCHA, 1)]
        schb_ap = cf_t[:, ds(CF_SCHB, 1)]

        kTb = [kt_pool.tile([128, CHUNK], BF16, name=f"kT_{u}", tag="kt")
               for u in range(UPC)]
        qTb = [qt_pool.tile([128, CHUNK], BF16, name=f"qT_{u}", tag="qt")
               for u in range(UPC)]
        kp = [kp_pool.tile([128, CHUNK], BF16, name=f"kp_{u}", tag="kp")
              for u in range(UPC)]

        def relu_store(dst, ps, bias_ap, eng):
            if eng == "A":
                nc.scalar.activation(dst, ps, mybir.ActivationFunctionType.Relu,
                                     bias=bias_ap)
            else:
                nc.vector.tensor_scalar(dst, ps, bias_ap, 0.0,
                                        op0=mybir.AluOpType.add,
                                        op1=mybir.AluOpType.max)

        def proj_k_half(u, half):
            """kTb[u] half = relu(wlid.T @ xlid + b), bf16."""
            psk = mm_psum.tile([128, QB], F32, name=f"psk_{u}_{half}",
                               tag="mm")
            for b2 in range(2):
                blk = half * 2 + b2
                for ci in range(NCI_L):
                    nc.tensor.matmul(
                        psk[:, ts(b2, 512)],
                        cb_t[:, ds(CB_WLID + ci * 128, 128)],
                        xl_ts[u][:, ci, ds(blk * 512, 512)],
                        start=(ci == 0), stop=(ci == NCI_L - 1))
            relu_store(kTb[u][:, ts(half, QB)], psk[:],
                       cf_t[:, ds(CF_LID_B, 1)], RELU_ENG[u * 4 + half])

        def transpose_group(u, g):
            pt = mm_psum.tile([128, 1024], BF16, name=f"pt_{u}_{g}", tag="mm")
            for k8 in range(8):
                nc.tensor.transpose(pt[:, ts(k8, 128)],
                                    kTb[u][:, ds(g * 1024 + k8 * 128, 128)],
                                    ident)
            nc.vector.tensor_copy(kp[u][:, ts(g, 1024)], pt[:])

        def proj_q_half(u, half):
            psq = mm_psum.tile([128, QB], F32, name=f"psq_{u}_{half}",
                               tag="mm")
            for b2 in range(2):
                blk = half * 2 + b2
                for ci in range(NCI_I):
                    nc.tensor.matmul(
                        psq[:, ts(b2, 512)],
                        cb_t[:, ds(CB_WIMG + ci * 128, 128)],
                        xi_ts[u][:, ci, ds(blk * 512, 512)],
                        start=(ci == 0), stop=(ci == NCI_I - 1))
            relu_store(qTb[u][:, ts(half, QB)], psq[:],
                       cf_t[:, ds(CF_IMG_B, 1)], RELU_ENG[u * 4 + 2 + half])

        def proj(u):
            # kT h0 then qT h0 first: scores of qb0 (k-slices 0-7) can
            # start while the rest of the projections are still going
            proj_k_half(u, 0)
            proj_q_half(u, 0)
            proj_k_half(u, 1)
            transpose_group(u, 0)
            transpose_group(u, 1)
            proj_q_half(u, 1)

        def attn(u, qb, inject=None, drain_fast=False):
            """Emit one q-block's attention; returns a drain closure that
            the caller emits later (inside the next q-block's slice loop)
            so the denominator matmuls never block the next block's
            scores in the PE queue.  `inject` maps slice index -> list of
            closures (deferred proj pieces / previous block's drain)."""
            lag = 1 if drain_fast else ADD_LAG
            po = av_psum.tile([128, QB], F32, name=f"po_{u}_{qb}", tag="av")
            # ping-pong S so DVE adds never read+write the same tile
            Ss = [s_pool.tile([128, QB], BF16, name=f"S{t}_{u}_{qb}", tag="S")
                  for t in range(2)]
            ets = [None] * KSL

            def s_add(i):
                # S_{i} = S_{i-1} + et_i  (i >= 1); S_1 = et_0 + et_1
                src0 = ets[0][:] if i == 1 else Ss[i % 2][:]
                eng = nc.vector if ADD_ENG[i] == "v" else nc.gpsimd
                eng.tensor_add(Ss[(i + 1) % 2][:], src0, ets[i][:])

            for i in range(KSL + max(LOOKAHEAD, lag + 1)):
                if inject and i in inject:
                    for fn in inject[i]:
                        fn()
                if i < KSL:
                    ps = mm_psum.tile([128, QB], F32,
                                      name=f"pss_{u}_{qb}_{i}", tag="mm")
                    for h in range(2):
                        nc.tensor.matmul(ps[:, ts(h, 512)],
                                         kTb[u][:, ds(i * 128, 128)],
                                         qTb[u][:, ds(qb * QB + h * 512, 512)],
                                         start=True, stop=True)
                    et = et_pool.tile([128, QB], BF16,
                                      name=f"et_{u}_{qb}_{i}", tag="et")
                    if EXP_ENG[i] == "A":
                        nc.scalar.activation(et[:], ps[:],
                                             mybir.ActivationFunctionType.Exp,
                                             scale=esc_ap)
                    else:
                        nc.vector.tensor_scalar(et[:].bitcast(I16), ps[:],
                                                scha_ap, schb_ap,
                                                op0=mybir.AluOpType.mult,
                                                op1=mybir.AluOpType.add)
                    ets[i] = et
                a = i - lag
                if 1 <= a < KSL:
                    s_add(a)
                j = i - LOOKAHEAD
                if 0 <= j < KSL:
                    for h in range(2):
                        nc.tensor.matmul(po[:, ts(h, 512)],
                                         kp[u][:, ds(j * 128, 128)],
                                         ets[j][:, ts(h, 512)],
                                         start=(j == 0), stop=(j == KSL - 1))

            def drain():
                S = Ss[KSL % 2]
                # denominator: PE broadcast-sum (1/w1)^T @ S, reciprocal,
                # then res = po * (w1/L) + qTb
                pl = mm_psum.tile([128, QB], F32, name=f"pl_{u}_{qb}",
                                  tag="mm")
                for h in range(2):
                    nc.tensor.matmul(pl[:, ts(h, 512)], invw1,
                                     S[:, ts(h, 512)], start=True, stop=True)
                nhalf = 2 if drain_fast else 1
                wd = QB // nhalf
                for h in range(nhalf):
                    linv = misc_pool.tile([128, wd], F32,
                                          name=f"linv_{u}_{qb}_{h}",
                                          tag="linv")
                    nc.vector.reciprocal_approx_fast(linv[:], pl[:, ts(h, wd)])
                    tmp = misc_pool.tile([128, wd], F32,
                                         name=f"tmp_{u}_{qb}_{h}", tag="tmp")
                    nc.vector.tensor_mul(tmp[:], po[:, ts(h, wd)], linv[:])
                    res = res_pool.tile([128, wd], F32,
                                        name=f"res_{u}_{qb}_{h}", tag="res")
                    nc.vector.tensor_add(res[:], tmp[:],
                                         qTb[u][:, ds(qb * QB + h * wd, wd)])
                    nc.sync.dma_start(y[u][:, ds(qb * QB + h * wd, wd)],
                                      res[:])
            return drain

        # software pipeline: first half-projections up front, the rest of
        # the projection work and each q-block's drain injected into the
        # following q-block's slice loop so PE/ACT/DVE never see a block
        # boundary bubble
        proj_k_half(0, 0)
        proj_q_half(0, 0)
        d00 = attn(0, 0, inject={
            1: [lambda: proj_k_half(0, 1)],
            2: [lambda: transpose_group(0, 0)],
            3: [lambda: transpose_group(0, 1)],
            4: [lambda: proj_q_half(0, 1)],
        })
        d01 = attn(0, 1, inject={
            1: [d00],
            2: [lambda: proj_k_half(1, 0)],
            3: [lambda: proj_q_half(1, 0)],
            4: [lambda: proj_k_half(1, 1)],
            5: [lambda: transpose_group(1, 0)],
            6: [lambda: transpose_group(1, 1)],
            7: [lambda: proj_q_half(1, 1)],
        })
        d10 = attn(1, 0, inject={1: [d01]})
        d11 = attn(1, 1, inject={1: [d10]}, drain_fast=True)
        d11()

    nc.compile()
    return nc


def _shard_inputs(inputs):
    """Build the 8 per-core input maps from the full input dict."""
    bf = mybir.dt.np(BF16)
    mw = np.asarray(inputs["modality_weights"], np.float64)
    e = np.exp(mw - mw.max())
    w = (e / e.sum()).astype(np.float64)
    w0, w1 = float(w[0]), float(w[1])

    def bn_fold(gamma, beta, mean, var, mul):
        g = np.asarray(gamma, np.float64)
        b = np.asarray(beta, np.float64)
        m = np.asarray(mean, np.float64)
        v = np.asarray(var, np.float64)
        scale = g / np.sqrt(v + EPS) * mul
        bias = (b - m * g / np.sqrt(v + EPS)) * mul
        return scale, bias

    i_s, i_b = bn_fold(inputs["image_gamma"], inputs["image_beta"],
                       inputs["image_mean"], inputs["image_var"], w0)
    l_s, l_b = bn_fold(inputs["lidar_gamma"], inputs["lidar_beta"],
                       inputs["lidar_mean"], inputs["lidar_var"], 1.0)

    # weight slices, pre-transposed for lhsT ([cin_slice, cout]), BN scale
    # folded in, bf16
    wi = (np.asarray(inputs["image_w"], np.float64).T * i_s[None, :])
    wl = (np.asarray(inputs["lidar_w"], np.float64).T * l_s[None, :])
    wi = wi.astype(np.float32).astype(bf).reshape(NCI_I, 128, CO)
    wl = wl.astype(np.float32).astype(bf).reshape(NCI_L, 128, CO)

    cb = np.zeros((128, 1024), bf)
    for ci in range(NCI_I):
        cb[:, CB_WIMG + ci * 128: CB_WIMG + (ci + 1) * 128] = wi[ci]
    for ci in range(NCI_L):
        cb[:, CB_WLID + ci * 128: CB_WLID + (ci + 1) * 128] = wl[ci]
    cb[:, CB_IDENT:CB_IDENT + 128] = np.eye(128, dtype=bf)
    cb[:, CB_INVW1:CB_INVW1 + 128] = np.full((128, 128), 1.0 / w1, bf)

    escv = 1.0 / (w0 * math.sqrt(CO))
    cfv = np.zeros((128, 8), np.float32)
    cfv[:, CF_IMG_B] = i_b.astype(np.float32)
    cfv[:, CF_LID_B] = l_b.astype(np.float32)
    cfv[:, CF_ESC] = escv
    cfv[:, CF_SCHA] = escv * SCH_A
    cfv[:, CF_SCHB] = 127.0 * 128.0 - SCH_C

    # features -> (B, C, nchunks, 2048) bf16
    img = np.asarray(inputs["image_features"], np.float32).astype(bf) \
        .reshape(B, NCI_I, 128, NCH, CHUNK)
    lid = np.asarray(inputs["lidar_features"], np.float32).astype(bf) \
        .reshape(B, NCI_L, 128, NCH, CHUNK)

    in_maps = []
    for core in range(NCORES):
        ximg = np.empty((UPC, 2, 128, NCI_I, 1024), bf)
        xlid = np.empty((UPC, 2, 128, NCI_L, 1024), bf)
        for ul in range(UPC):
            un = core * UPC + ul
            b, c = un // NCH, un % NCH
            for h in range(2):
                # [ci, 128, 1024] -> [128, ci, 1024]
                ximg[ul, h] = img[b, :, :, c, h * 1024:(h + 1) * 1024] \
                    .transpose(1, 0, 2)
                xlid[ul, h] = lid[b, :, :, c, h * 1024:(h + 1) * 1024] \
                    .transpose(1, 0, 2)
        in_maps.append({"cb": cb, "cf": cfv, "xl": xlid, "xi": ximg})
    return in_maps


def kernel(**inputs) -> np.ndarray:
    global _PROGRAM, LAST_RESULTS
    if _PROGRAM is None:
        _PROGRAM = _build_program()
    nc = _PROGRAM

    in_maps = _shard_inputs(inputs)
    trace = os.environ.get("BASS_KERNEL_TRACE", "0") == "1"
    tmpdir = os.environ.get("BASS_KERNEL_TRACE_DIR") or None
    if tmpdir:
        os.makedirs(tmpdir, exist_ok=True)
    results = run_bass_kernel_spmd(nc, in_maps, core_ids=list(range(NCORES)),
                                   trace=trace, tmpdir=tmpdir)
    LAST_RESULTS = results

    out = np.empty((B, CO, H, W), np.float32)
    outv = out.reshape(B, CO, NCH, CHUNK)
    for core in range(NCORES):
        yc = results.results[core]["y"]
        for ul in range(UPC):
            un = core * UPC + ul
            b, c = un // NCH, un % NCH
            outv[b, :, c, :] = yc[ul]
    return out


if __name__ == "__main__":
    rng = np.random.default_rng(0)
    inputs = {
        "lidar_features": rng.standard_normal((B, CL, H, W), np.float32),
        "image_features": rng.standard_normal((B, CI, H, W), np.float32),
        "lidar_w": rng.standard_normal((CO, CL), np.float32) * np.sqrt(2.0 / CO),
        "lidar_gamma": np.ones(CO, np.float32),
        "lidar_beta": np.zeros(CO, np.float32),
        "lidar_mean": rng.standard_normal(CO).astype(np.float32) * 0.1,
        "lidar_var": rng.uniform(0.5, 1.5, CO).astype(np.float32),
        "image_w": rng.standard_normal((CO, CI), np.float32) * np.sqrt(2.0 / CO),
        "image_gamma": np.ones(CO, np.float32),
        "image_beta": np.zeros(CO, np.float32),
        "image_mean": rng.standard_normal(CO).astype(np.float32) * 0.1,
        "image_var": rng.uniform(0.5, 1.5, CO).astype(np.float32),
        "modality_weights": np.ones(2, np.float32),
    }
    out = kernel(**inputs)
    print("kernel out:", out.shape, out.dtype, float(np.abs(out).mean()))
